# revision 2
# baseline (speedup 1.0000x reference)
"""BERT self-attention on 8 Trainium2 NeuronCores.

Sharding: data-parallel over batch (B=8 -> 1 batch element per core).
Every core runs the same single-core Bass kernel on its own batch slice;
weights/mask are replicated. The final output is a host-side stack.

Per-core algorithm (S=1024, HID=1024, NH=16, HD=64), all matmuls bf16
with fp32 PSUM accumulation:

  xT = X^T (host-transposed, bf16)             [HID, S]
  Q^T = Wq^T @ X^T   (lhsT = Wq col-chunks)    [HID, S]  (+bq per-partition)
  K^T = Wk^T @ X^T                             [HID, S]  (+bk per-partition)
  V   = X @ Wv       (lhsT = xT)               [S, HID]  (+bv broadcast)
  per head pair (2c, 2c+1) living in hid chunk c:
    S^T = K_h @ Q_h^T  via TWO concurrent 64-contraction matmuls at PE
          tile_position (0,0) / (64,0) -- the two heads' K/Q rows live in
          partition halves 0:64 / 64:128 of chunk c, so the PE row-groups
          run both heads simultaneously (no zero-padding, no half-wasted
          contraction rows).
    P^T = exp(S^T/8 + mask[k])   (ScalarE; this engine is the kernel
          bottleneck: 128 exps x ~1.1us = 142us of ACT time. The whole
          schedule is built to start this stream as early as possible and
          keep it dense.)
    ctx = P^T.T @ [V_h | 1]  (ones column yields the softmax denominator)
    out[:, h] = ctx[:, :64] * (1/Z)

Pipeline order: chunk 0's Q/K come first (wq/wk are DMA'd as column
chunks so Q(0)/K(0) only wait for xT + one 256KB column slice), scores+
exp(0) start ~ics earliest; V projections are emitted after chunk 1 and
fill PE slack under the ACT-bound phase; ctx for chunk c runs during
chunk c+2's exp window.
"""

import functools

import numpy as np
import ml_dtypes

B, S, HID = 8, 1024, 1024
NH, HD = 16, 64
P = 128
NCH = HID // P  # hid chunks (8)
NKT = S // P  # key tiles (8)
NQT = S // P  # query tiles (8)
VROW = NH * (HD + 1)  # 1040: per-seq-chunk V row: 16 x (64 V cols + ones col)
N_CORES = 8

SCALE = 1.0 / float(np.sqrt(HD))


@functools.lru_cache(maxsize=None)
def _build(has_bv: bool):
    import concourse.bass as bass
    import concourse.tile as tile
    from concourse import bacc, mybir
    from contextlib import ExitStack

    fp32 = mybir.dt.float32
    bf16 = mybir.dt.bfloat16
    EXP = mybir.ActivationFunctionType.Exp

    nc = bacc.Bacc("TRN2", target_bir_lowering=False)

    xT = nc.dram_tensor("xT", [HID, S], bf16, kind="ExternalInput")
    # wq/wk host-pre-shuffled to [c_out, p(hid_in%128), kc(hid_in//128), col]
    # so one contiguous DMA delivers the full column chunk c (everything
    # Q(c)/K(c) need), letting chunk 0's projections start ~11us earlier.
    wq = nc.dram_tensor("wq", [NCH, P, NCH, P], bf16, kind="ExternalInput")
    wk = nc.dram_tensor("wk", [NCH, P, NCH, P], bf16, kind="ExternalInput")
    wv = nc.dram_tensor("wv", [HID, HID], bf16, kind="ExternalInput")
    bq = nc.dram_tensor("bq", [P, NCH], fp32, kind="ExternalInput")
    bk = nc.dram_tensor("bk", [P, NCH], fp32, kind="ExternalInput")
    bv = nc.dram_tensor("bv", [HID], fp32, kind="ExternalInput") if has_bv else None
    mask = nc.dram_tensor("mask", [P, NKT], fp32, kind="ExternalInput")
    out = nc.dram_tensor("out", [S, HID], fp32, kind="ExternalOutput")

    with tile.TileContext(nc) as tc, ExitStack() as ctx:
        persist = ctx.enter_context(tc.tile_pool(name="persist", bufs=1))
        misc = ctx.enter_context(tc.tile_pool(name="misc", bufs=8))
        pT_pool = ctx.enter_context(tc.tile_pool(name="pT", bufs=4))
        out_pool = ctx.enter_context(tc.tile_pool(name="out", bufs=2))
        qkv_ps = ctx.enter_context(tc.tile_pool(name="qkv_ps", bufs=2, space="PSUM"))
        sc_ps = ctx.enter_context(tc.tile_pool(name="sc_ps", bufs=2, space="PSUM"))
        cx_ps = ctx.enter_context(tc.tile_pool(name="cx_ps", bufs=2, space="PSUM"))

        # ---- persistent SBUF tensors ----
        xT_c = [persist.tile([P, S], bf16, name=f"xT{c}") for c in range(NCH)]
        wq_c = [persist.tile([P, NCH, P], bf16, name=f"wq{c}") for c in range(NCH)]
        wk_c = [persist.tile([P, NCH, P], bf16, name=f"wk{c}") for c in range(NCH)]
        wv_c = [persist.tile([P, HID], bf16, name=f"wv{c}") for c in range(NCH)]
        qT_sb = persist.tile([P, NCH, S], bf16)  # [p, hidout_chunk, seq]
        kT_sb = persist.tile([P, NCH, S], bf16)  # head 2c: part 0:64, 2c+1: 64:128
        v_sb = persist.tile([P, NKT, VROW], bf16)  # [p(seq), seq_chunk, 16*(64+1)]
        bq_sb = persist.tile([P, NCH], fp32)
        bk_sb = persist.tile([P, NCH], fp32)
        mask_sb = persist.tile([P, NKT], fp32)
        bv_sb = persist.tile([P, HID], fp32, name="bv_sb") if has_bv else None

        # ---- input DMAs, latency-ordered ----
        nc.sync.dma_start(out=bq_sb, in_=bq[:, :])
        nc.sync.dma_start(out=bk_sb, in_=bk[:, :])
        nc.sync.dma_start(out=mask_sb, in_=mask[:, :])
        if has_bv:
            bv_bcast = bass.AP(tensor=bv.tensor if hasattr(bv, "tensor") else bv,
                               offset=0, ap=[[0, P], [1, HID]])
            nc.sync.dma_start(out=bv_sb, in_=bv_bcast)
        for c in range(NCH):
            nc.sync.dma_start(out=xT_c[c], in_=xT[c * P:(c + 1) * P, :])
        # chunk-c weight columns in chunk order: Q(0)/K(0) unblock first
        for c in range(NCH):
            nc.sync.dma_start(out=wq_c[c], in_=wq[c])
            nc.sync.dma_start(out=wk_c[c], in_=wk[c])
        for c in range(NCH):
            nc.sync.dma_start(out=wv_c[c], in_=wv[c * P:(c + 1) * P, :])

        # ones columns for the softmax denominator live at col 64 of each
        # 65-wide head block; V copies below only overwrite cols 0..63
        nc.gpsimd.memset(v_sb, 1.0)

        # warmup matmuls on scratch data while the input DMAs stream in:
        # keeps the PE busy so the HAM clock-gate reaches 8/8 before real
        # work arrives (otherwise the first ~3.4us of matmuls run slow)
        wscr = persist.tile([P, 512], bf16, name="warm_scratch")
        nc.vector.memset(wscr, 0.5)
        for _ in range(16):
            wps = sc_ps.tile([P, S], fp32, name="score_psum")
            nc.tensor.matmul(
                wps[:, 0:512],
                lhsT=wscr[:, 0:P],
                rhs=wscr,
                start=True,
                stop=True,
            )

        def qk_proj(c, w_c, b_sb, dst_sb):
            # dst[:, c, :] = (W^T @ X^T + b) for hid-out chunk c
            for half in range(2):
                ps = qkv_ps.tile([P, 512], fp32, name="qkv_psum")
                for kc in range(NCH):
                    nc.tensor.matmul(
                        ps,
                        lhsT=w_c[c][:, kc, :],
                        rhs=xT_c[kc][:, half * 512:(half + 1) * 512],
                        start=(kc == 0),
                        stop=(kc == NCH - 1),
                    )
                nc.vector.tensor_scalar_add(
                    out=dst_sb[:, c, half * 512:(half + 1) * 512],
                    in0=ps,
                    scalar1=b_sb[:, c:c + 1],
                )

        def v_proj(st):
            # v_sb[:, st, heads] = X @ Wv (+bv) for seq chunk st
            for half in range(2):
                ps = qkv_ps.tile([P, 512], fp32, name="qkv_psum")
                for kc in range(NCH):
                    nc.tensor.matmul(
                        ps,
                        lhsT=xT_c[kc][:, st * P:(st + 1) * P],
                        rhs=wv_c[kc][:, half * 512:(half + 1) * 512],
                        start=(kc == 0),
                        stop=(kc == NCH - 1),
                    )
                dst = (
                    v_sb[:, st, :]
                    .rearrange("p (h x) -> p h x", x=HD + 1)[:, half * 8:(half + 1) * 8, 0:HD]
                )
                src = ps.rearrange("p (h x) -> p h x", x=HD)
                if has_bv:
                    bvs = (
                        bv_sb[:, half * 512:(half + 1) * 512]
                        .rearrange("p (h x) -> p h x", x=HD)
                    )
                    nc.vector.tensor_add(out=dst, in0=src, in1=bvs)
                else:
                    nc.vector.tensor_copy(out=dst, in_=src)

        def scores_exp(c):
            # S^T then P^T for heads 2c (partitions 0:64) and 2c+1 (64:128).
            # The two heads' matmuls run CONCURRENTLY on the PE via row-group
            # tiling: 64-contraction tiles at tile_position (0,0) and (64,0)
            # (auto-derived from the operands' base partitions).
            pT_A = pT_pool.tile([P, NKT, S], bf16, name="pT")
            pT_B = pT_pool.tile([P, NKT, S], bf16, name="pT")
            for kt in range(NKT):
                ps_A = sc_ps.tile([P, S], fp32, name="score_psum")
                ps_B = sc_ps.tile([P, S], fp32, name="score_psum")
                for half in range(2):
                    nc.tensor.matmul(
                        ps_A[:, half * 512:(half + 1) * 512],
                        lhsT=kT_sb[0:64, c, kt * P:(kt + 1) * P],
                        rhs=qT_sb[0:64, c, half * 512:(half + 1) * 512],
                        start=True,
                        stop=True,
                    )
                    nc.tensor.matmul(
                        ps_B[:, half * 512:(half + 1) * 512],
                        lhsT=kT_sb[64:128, c, kt * P:(kt + 1) * P],
                        rhs=qT_sb[64:128, c, half * 512:(half + 1) * 512],
                        start=True,
                        stop=True,
                    )
                # P^T = exp(scores/8 + mask_k); bf16 out, straight to SBUF
                nc.scalar.activation(
                    out=pT_A[:, kt, :],
                    in_=ps_A,
                    func=EXP,
                    bias=mask_sb[:, kt:kt + 1],
                    scale=SCALE,
                )
                nc.scalar.activation(
                    out=pT_B[:, kt, :],
                    in_=ps_B,
                    func=EXP,
                    bias=mask_sb[:, kt:kt + 1],
                    scale=SCALE,
                )
            return pT_A, pT_B

        def ctx_head(h, pT_h):
            head_out = out_pool.tile([P, NQT, HD], fp32, name="head_out")
            for qt in range(NQT):
                cps = cx_ps.tile([P, HD + 1], fp32, name="ctx_psum")
                for kc in range(NKT):
                    nc.tensor.matmul(
                        cps,
                        lhsT=pT_h[:, kc, qt * P:(qt + 1) * P],
                        rhs=v_sb[:, kc, h * (HD + 1):(h + 1) * (HD + 1)],
                        start=(kc == 0),
                        stop=(kc == NKT - 1),
                    )
                recip = misc.tile([P, 1], fp32, name="recip")
                nc.vector.reciprocal(recip, cps[:, HD:HD + 1])
                nc.vector.tensor_scalar_mul(
                    out=head_out[:, qt, :],
                    in0=cps[:, 0:HD],
                    scalar1=recip,
                )
            # stream this head's output columns out while later heads run
            for qt in range(NQT):
                nc.sync.dma_start(
                    out=out[qt * P:(qt + 1) * P, h * HD:(h + 1) * HD],
                    in_=head_out[:, qt, :],
                )

        # ---- pipeline ----
        # exp stream starts with chunk 0; V fills PE slack under chunk 1's
        # exps; ctx for chunk c runs under chunk c+2's exp window.
        pT_live = {}
        for c in range(NCH):
            qk_proj(c, wq_c, bq_sb, qT_sb)
            qk_proj(c, wk_c, bk_sb, kT_sb)
            pT_live[c] = scores_exp(c)
            if c == 1:
                for st in range(NKT):
                    v_proj(st)
            if c >= 2:
                pA, pB = pT_live.pop(c - 2)
                ctx_head(2 * (c - 2), pA)
                ctx_head(2 * (c - 2) + 1, pB)
        for c in (NCH - 2, NCH - 1):
            pA, pB = pT_live.pop(c)
            ctx_head(2 * c, pA)
            ctx_head(2 * c + 1, pB)

    nc.finalize()
    return nc


def _prep_inputs(inputs):
    bf16 = ml_dtypes.bfloat16
    hs = np.asarray(inputs["hidden_states"], dtype=np.float32)
    am = np.asarray(inputs["attention_mask"], dtype=np.float32)
    Wq = np.asarray(inputs["Wq"], dtype=np.float32)
    Wk = np.asarray(inputs["Wk"], dtype=np.float32)
    Wv = np.asarray(inputs["Wv"], dtype=np.float32)
    bq = np.asarray(inputs["bq"], dtype=np.float32)
    bk = np.asarray(inputs["bk"], dtype=np.float32)
    bv = np.asarray(inputs["bv"], dtype=np.float32)

    has_bv = bool(np.any(bv))

    # [hid_in, hid_out] -> [c_out, p(hid_in%128), kc(hid_in//128), col]
    def col_shuffle(w):
        return np.ascontiguousarray(
            w.astype(bf16).reshape(NCH, P, NCH, P).transpose(2, 1, 0, 3)
        )

    wq_b = col_shuffle(Wq)
    wk_b = col_shuffle(Wk)
    wv_b = np.ascontiguousarray(Wv.astype(bf16))
    bq_c = np.ascontiguousarray(bq.reshape(NCH, P).T)
    bk_c = np.ascontiguousarray(bk.reshape(NCH, P).T)

    hs_b = hs.astype(bf16)
    in_maps = []
    for b in range(B):
        m = {
            "xT": np.ascontiguousarray(hs_b[b].T),
            "wq": wq_b,
            "wk": wk_b,
            "wv": wv_b,
            "bq": bq_c,
            "bk": bk_c,
            "mask": np.ascontiguousarray(am[b, 0, 0].reshape(NKT, P).T),
        }
        if has_bv:
            m["bv"] = bv
        in_maps.append(m)
    return in_maps, has_bv


def _run(inputs, trace=False, trace_cores=None):
    from concourse.bass_utils import run_bass_kernel_spmd

    in_maps, has_bv = _prep_inputs(inputs)
    nc = _build(has_bv)
    res = run_bass_kernel_spmd(
        nc, in_maps, core_ids=list(range(N_CORES)), trace=trace,
        trace_cores=trace_cores,
    )
    out = np.stack([np.asarray(r["out"], dtype=np.float32) for r in res.results])
    return out, res


def kernel(**inputs) -> np.ndarray:
    out, _ = _run(inputs, trace=False)
    return out
